# revision 1
# baseline (speedup 1.0000x reference)
"""ConsensusAttention Trainium2 kernel.

Full-input contract: kernel(levels, non_local_mask) -> out, shapes
  levels:         (8, 1024, 6, 512) float32
  non_local_mask: (1024, 1024) bool   (True = masked out)
  out:            (8, 1024, 6, 512) float32

Sharding: data-parallel over batch (8 cores, one batch element each).

Math per batch element, per level l:
  X = levels[:, l, :]                        (n=1024, d=512)
  r[j] = 1 / (sqrt(d) * ||X_j||)
  S[i, j] = <X_i, X_j> * r[j]
  A = softmax_j(S masked)                    (mask is a local-window mask)
  out[:, l, :] = A @ X

The mask only admits keys with |j - i| <= 96, so for each 256-query
superblock q only key-tiles 2q-1..2q+2 (128 wide, clamped to [0,7])
participate. Scores are computed transposed (S^T[j, i]) so the per-key
scale r[j] rides the ACT exp as a per-partition scale and the exp tiles
feed the output matmul directly as stationary operands (no attention
transposes). Scores are O(1) (|S| <= ||X_i||/sqrt(d) ~ 1.1) so softmax
needs no max-shift; masking is an exact multiply by a 0/1 mask after
exp. Row sums ride N=2 ones-matmuls into PSUM.

Matmuls run in float32r (~tf32, full PE rate at moving dim >= 256; HW
requires producers to write f32r-typed outputs — the DRAM input is
declared f32r (bit-identical) and the transpose/mask copies round).
"""

import sys

sys.path.insert(0, "/opt/trn_rl_repo")

import numpy as np

import concourse.bacc as bacc
import concourse.tile as tile
from concourse import mybir
from concourse.masks import make_identity
from concourse.bass_utils import run_bass_kernel_spmd

B, N, L, D = 8, 1024, 6, 512
NT = N // 128   # 8 key tiles
DC = D // 128   # 4 contraction chunks
NQ = 4          # 256-query superblocks
F32 = mybir.dt.float32
F32R = mybir.dt.float32r


def _tiles(q):
    # key tiles with any unmasked entry for query superblock q
    return list(range(max(2 * q - 1, 0), min(2 * q + 2, NT - 1) + 1))


def _jlo(q):
    # start tile of the (up to 512-wide) mask window staged for q
    return min(max(2 * q - 1, 0), NT - 4)





def _build_nc():
    nc = bacc.Bacc(
        "TRN2",
        target_bir_lowering=False,
        debug=False,
        enable_asserts=True,
        num_devices=8,
    )
    # lv is declared f32r: bit-identical to the f32 numpy input, and lets the
    # DMA land X directly in matmul-legal tiles (PE rounds on read).
    lv = nc.dram_tensor("lv", [N, L, D], F32R, kind="ExternalInput").ap()
    m01 = nc.dram_tensor(
        "m01", [NQ, 512, 256], mybir.dt.bfloat16, kind="ExternalInput"
    ).ap()
    out = nc.dram_tensor("out", [N, L, D], F32, kind="ExternalOutput").ap()

    with tile.TileContext(nc) as tc:
        with (
            tc.tile_pool(name="singles", bufs=1) as singles,
            tc.tile_pool(name="xn_p", bufs=3) as xn_p,
            tc.tile_pool(name="xt_p", bufs=2) as xt_p,
            tc.tile_pool(name="sq_p", bufs=4) as sq_p,
            tc.tile_pool(name="r_p", bufs=2) as r_p,
            tc.tile_pool(name="small_p", bufs=8) as small_p,
            tc.tile_pool(name="e0_p", bufs=6) as e0_p,
            tc.tile_pool(name="et_p", bufs=14) as et_p,
            tc.tile_pool(name="ob_p", bufs=4) as ob_p,
            tc.tile_pool(name="pt_p", bufs=3, space="PSUM") as pt_p,
            tc.tile_pool(name="ps_p", bufs=3, space="PSUM") as ps_p,
            tc.tile_pool(name="po_p", bufs=1, space="PSUM") as po_p,
            tc.tile_pool(name="ss_p", bufs=1, space="PSUM") as ss_p,
        ):
            ident = singles.tile([128, 128], F32)
            make_identity(nc, ident)
            ones_f32 = singles.tile([128, 2], F32)
            nc.vector.memset(ones_f32, 1.0)
            ones2 = singles.tile([128, 2], F32R)
            nc.scalar.copy(out=ones2, in_=ones_f32)
            m01_sb = singles.tile([128, NQ, 4, 256], mybir.dt.bfloat16)

            for l in range(L):
                xn = xn_p.tile([128, NT, D], F32R)
                for c in range(NT):
                    nc.sync.dma_start(
                        out=xn[:, c, :],
                        in_=lv[c * 128 : (c + 1) * 128, l, :],
                    )

                # r[j] = 1/sqrt(D * sum(X_j^2)), one column per key tile
                # (square on the otherwise-idle GPSIMD, reduce on DVE)
                rt = r_p.tile([128, NT], F32)
                r_all = r_p.tile([128, NT], F32)
                if l == 0:
                    # after the level-0 X loads so they win the DMA engines
                    nc.sync.dma_start(
                        out=m01_sb, in_=m01.rearrange("q (t p) i -> p q t i", p=128)
                    )
                nrm = r_p.tile([128, NT], F32)
                for jt in range(NT):
                    sq = sq_p.tile([128, D], F32)
                    nc.gpsimd.tensor_mul(out=sq, in0=xn[:, jt, :], in1=xn[:, jt, :])
                    nc.vector.reduce_sum(
                        out=rt[:, jt : jt + 1], in_=sq, axis=mybir.AxisListType.X
                    )
                nc.scalar.activation(
                    out=nrm, in_=rt, func=mybir.ActivationFunctionType.Sqrt,
                    scale=float(D),
                )
                nc.vector.reciprocal(out=r_all, in_=nrm)

                # X^T via PE transposes: xt[pd, dc, j] = X[j, dc*128+pd].
                # 4 dc-chunks share one PSUM bank; one batched copy per tile.
                xt = xt_p.tile([128, DC, N], F32R)
                for jt in range(NT):
                    pt = pt_p.tile([128, DC, 128], F32)
                    for dc in range(DC):
                        nc.tensor.transpose(
                            out=pt[:, dc, :],
                            in_=xn[:, jt, dc * 128 : (dc + 1) * 128].bitcast(F32),
                            identity=ident,
                        )
                    dst = xt[:, :, jt * 128 : (jt + 1) * 128]
                    if jt % 4 == 0:
                        nc.scalar.copy(out=dst, in_=pt)
                    else:
                        nc.vector.tensor_copy(out=dst, in_=pt)

                for q in range(NQ):
                    jlo = _jlo(q)
                    tl = _tiles(q)
                    qs = slice(q * 256, (q + 1) * 256)
                    ets = {}
                    for jt in tl:
                        ps = ps_p.tile([128, 256], F32)
                        for dc in range(DC):
                            nc.tensor.matmul(
                                ps,
                                lhsT=xt[:, dc, jt * 128 : (jt + 1) * 128],
                                rhs=xt[:, dc, qs],
                                start=(dc == 0),
                                stop=(dc == DC - 1),
                            )
                        e0 = e0_p.tile([128, 256], F32)
                        nc.scalar.activation(
                            out=e0,
                            in_=ps,
                            func=mybir.ActivationFunctionType.Exp,
                            scale=r_all[:, jt : jt + 1],
                        )
                        et = et_p.tile([128, 256], F32R)
                        nc.vector.tensor_mul(
                            out=et, in0=e0, in1=m01_sb[:, q, jt - jlo, :]
                        )
                        ets[jt] = et

                    ss = ss_p.tile([128, 4], F32)
                    ob = ob_p.tile([128, 2, D], F32)
                    for h in range(2):
                        po = po_p.tile([128, D], F32)
                        for i, jt in enumerate(tl):
                            eh = ets[jt][:, h * 128 : (h + 1) * 128]
                            nc.tensor.matmul(
                                po,
                                lhsT=eh,
                                rhs=xn[:, jt, :],
                                start=(i == 0),
                                stop=(i == len(tl) - 1),
                            )
                            nc.tensor.matmul(
                                ss[:, 2 * h : 2 * h + 2],
                                lhsT=eh,
                                rhs=ones2,
                                start=(i == 0),
                                stop=(i == len(tl) - 1),
                            )
                        rec = small_p.tile([128, 1], F32)
                        nc.vector.reciprocal(out=rec, in_=ss[:, 2 * h : 2 * h + 1])
                        if h == 0:
                            nc.scalar.activation(
                                out=ob[:, 0, :],
                                in_=po,
                                func=mybir.ActivationFunctionType.Copy,
                                scale=rec,
                            )
                        else:
                            nc.vector.tensor_scalar_mul(
                                out=ob[:, 1, :], in0=po, scalar1=rec
                            )
                    for h2 in range(2):
                        nc.sync.dma_start(
                            out=out[q * 256 + h2 * 128 : q * 256 + (h2 + 1) * 128, l, :],
                            in_=ob[:, h2, :],
                        )

    nc.compile()
    return nc


_NC = None


def get_nc():
    global _NC
    if _NC is None:
        _NC = _build_nc()
    return _NC


def _band_ok(mask):
    # every unmasked (i, j) must fall inside q's staged key tiles
    for q in range(NQ):
        rows = ~mask[q * 256 : (q + 1) * 256, :]
        outside = np.ones(N, dtype=bool)
        for jt in _tiles(q):
            outside[jt * 128 : (jt + 1) * 128] = False
        if rows[:, outside].any():
            return False
    # no all-masked row (softmax denominator would be 0)
    if (~mask).sum(axis=1).min() == 0:
        return False
    return True


def _numpy_ref(levels, mask):
    levels = levels.astype(np.float32)
    nrm = np.linalg.norm(levels, axis=-1, keepdims=True)
    k = levels / np.maximum(nrm, 1e-12)
    sim = np.einsum("bild,bjld->blij", levels, k) * (levels.shape[-1] ** -0.5)
    sim = np.where(mask[None, None, :, :], -np.finfo(np.float32).max, sim)
    sim = sim - sim.max(axis=-1, keepdims=True)
    e = np.exp(sim)
    attn = e / e.sum(axis=-1, keepdims=True)
    return np.einsum("blij,bjld->bild", attn, levels).astype(np.float32)


def kernel(levels, non_local_mask):
    levels = np.ascontiguousarray(levels, dtype=np.float32)
    mask = np.asarray(non_local_mask).astype(bool)
    if levels.shape != (B, N, L, D) or mask.shape != (N, N) or not _band_ok(mask):
        return _numpy_ref(levels, mask)

    m01 = np.zeros((NQ, 512, 256), dtype=np.float32)
    for q in range(NQ):
        jlo = _jlo(q)
        w = (~mask[q * 256 : (q + 1) * 256, jlo * 128 : jlo * 128 + 512]).T
        m01[q] = w.astype(np.float32)

    import ml_dtypes

    m01 = m01.astype(ml_dtypes.bfloat16)
    nc = get_nc()
    in_maps = [{"lv": levels[b], "m01": m01} for b in range(B)]
    res = run_bass_kernel_spmd(nc, in_maps, core_ids=list(range(B)))
    return np.stack([res.results[b]["out"] for b in range(B)])

